# revision 9
# baseline (speedup 1.0000x reference)
"""Trainium2 Bass kernel for nn_Eq1to3 (eset_ops_1_to_3 + einsum broadcast expansion).

Reference computation (N=16, D=64, S=32, M=48, BASIS=4):
    t[b,n,s,m] = sum_d coefs[d,s,b] * x[n,d,m]        # tiny einsum
    out[n,s,i,j,k] = t0[n,s,i] + t1[n,s,j] + t2[n,s,k]
                     + (i==j==k) * t3[n,s,i] + bias[s]
Output (16, 32, 48, 48, 48) f32 = 226.5 MB -> HBM-write-bound.
Measured: each HWDGE ring sustains ~220 GB/s on full 128-partition
transfers; both rings together ~440 GB/s per core, so the per-core stream
floor is 28.3 MB / 440 GB/s ~= 65 us, plus ~10 us fixed startup (Tile
preamble + load-completion receipt) and a ~2.5 us tail. (64-partition
half-DMAs only engage 8 of the 16 SDMA engines and halve the ring rate;
partition-strided APs are rejected by the BIR verifier -- both measured.)

Strategy: data-parallel over N across 8 cores (2 batches/core). Per core the
output is [3072 rows p=(n,s,i), 2304 cols (j,k)]. Partition q holds the 24
consecutive rows p = 24*q + r, so ns(q) = q//2 and i(q,r) = 24*(q%2) + r.

All tables are produced by the TENSOR ENGINE in bf16 (single-pass matmuls,
fp32 PSUM accumulate; rel err ~2e-3 << the 2e-2 gate) from host-prepared
indicator weights lhsT[(n',d), q] = coefs[d, s(q), b] * (n'==n(q)) against
rhs slices/broadcasts of x2[(n,d), m] = x[n,d,m]:

    B_ps[q, (r,j)]      = t0[ns(q), i(q,r)] + t1[ns(q), j]  (3 matmuls/chunk)
    T2_ps[q, k]         = t2[ns(q), k]                       (1 matmul)
    T3_ps[q, 24*li + r] = t3[ns(q), 24*li+r] * (q%2 == li)   (2 l-masked
                          matmuls kept in SEPARATE column halves)

T2G = T2_ps + bias[s(q)] (one tiny DVE add folds the bias; no bias matmuls).
Main loop per piece of rows [r0, r0+rw): DVE (or GpSimd for a few pieces,
with an ACT PSUM->SBUF mirror of B) computes
out[q,(u,j,k)] = B[q,(u,j)] + T2G[q,k] via stride-0 broadcast APs. The
superdiagonal needs out[q, u*2304 + i*49] += t3 at the parity-dependent
i = 24*(q%2)+r0+u, which no legal AP can address directly; instead BOTH
candidate positions get a tiny stepped-AP add on all 128 partitions
(free offsets u*2353 + (24*li + r0)*49, li = 0,1) with values from the
l-MASKED T3G halves -- the zeros cancel the wrong-parity position, so no
one-hot mask tensor, no 590KB mask load, no GpSimd DGM multiply. One
contiguous 128-partition DMA per piece (rows 24q+r0 .. +rw per partition),
alternating the SP/ACT HWDGE rings; first and last rows go out as
single-row pieces so the stream starts early and ends with small transfers.
"""

import numpy as np

N, D, S, M, BASIS = 16, 64, 32, 48, 4
N_CORES = 8
NL = N // N_CORES              # batches per core (2)
NS = NL * S                    # (n,s) groups per core (64)
ROWS = NS * M                  # output rows per core (3072)
JK = M * M                     # free size per row (2304)
P = 128                        # partitions
HALF = M // 2                  # rows per partition (24)
K = NL * D                     # contraction size (128)
DSTEP = JK + M + 1             # free step between diag elems of adjacent rows

# B-matmul chunks in r-rows; each chunk gets its own PSUM bank (a PE-write
# and a DVE-read in the same PSUM bank is a hardware fault, so chunks must
# not share banks while the main loop streams from an earlier chunk).
# 6 B chunks + T2 + T3 = 8 PSUM tiles = all 8 banks.
CHUNKS = [1, 1, 2, 4, 8, 8]
CHUNK_R0 = [sum(CHUNKS[:c]) for c in range(len(CHUNKS))]

# critical-load column blocks (bf16): 0=t1, 1=t2, 2=t0l0, 3=t0l1, then x2
WC_COLS = 4 * P + M            # [K, 560]
# second load: 0=t3l0, 1=t3l1 (bf16)
WT_COLS = 2 * P

_PROG = None


def _build_prog():
    import concourse.bacc as bacc
    import concourse.tile as tile
    import concourse.mybir as mybir

    f32 = mybir.dt.float32
    bf16 = mybir.dt.bfloat16
    nc = bacc.Bacc("TRN2", target_bir_lowering=False, debug=False,
                   num_devices=N_CORES)

    wc_d = nc.dram_tensor("wc", [K, WC_COLS], bf16,
                          kind="ExternalInput").ap()
    wt_d = nc.dram_tensor("wt", [K, WT_COLS], bf16,
                          kind="ExternalInput").ap()
    bv_d = nc.dram_tensor("bv", [P, 1], f32, kind="ExternalInput").ap()
    y_d = nc.dram_tensor("y", [ROWS, JK], f32, kind="ExternalOutput").ap()

    with tile.TileContext(nc) as tc:
        with (
            tc.tile_pool(name="const", bufs=1) as cpool,
            tc.tile_pool(name="psum", bufs=1, space="PSUM") as ppool,
            tc.tile_pool(name="outp", bufs=8) as opool,
            tc.tile_pool(name="bsb", bufs=3) as bpool,
        ):
            # ---- loads: critical weights on SP ring, the rest on ACT ring
            wc_sb = cpool.tile([K, WC_COLS], bf16)
            nc.sync.dma_start(out=wc_sb[:], in_=wc_d[:])
            bv_sb = cpool.tile([P, 1], f32)
            nc.scalar.dma_start(out=bv_sb[:], in_=bv_d[:])
            wt_sb = cpool.tile([K, WT_COLS], bf16)
            nc.scalar.dma_start(out=wt_sb[:], in_=wt_d[:])

            w1_l = wc_sb[:, 0 * P:1 * P]
            w2_l = wc_sb[:, 1 * P:2 * P]
            w0_l = lambda li: wc_sb[:, (2 + li) * P:(3 + li) * P]
            x2_sb = wc_sb[:, 4 * P:4 * P + M]
            w3_l = lambda li: wt_sb[:, li * P:(li + 1) * P]

            # ---- T2 first (gates every output piece) ----
            T2_ps = ppool.tile([P, M], f32)
            nc.tensor.matmul(T2_ps[:], w2_l, x2_sb, start=True, stop=True)
            T2G = cpool.tile([P, M], f32)
            # fold bias in while copying PSUM->SBUF
            nc.vector.tensor_add(out=T2G[:], in0=T2_ps[:],
                                 in1=bv_sb[:, 0:1].broadcast_to((P, M)))

            # ---- B[q, (r, j)] via accumulating matmuls ----
            B_chunks = [ppool.tile([P, ci * M], f32, name=f"B_ps{c}")
                        for c, ci in enumerate(CHUNKS)]

            def emit_b_chunk(c):
                ci = CHUNKS[c]
                i0 = CHUNK_R0[c]
                blk = B_chunks[c].rearrange("q (r j) -> q r j", j=M)
                # t1 part: rhs[(n'd), (r, j)] = x[n', d, j]
                rhs = x2_sb[:, None, :].broadcast_to((K, ci, M))
                nc.tensor.matmul(blk, w1_l, rhs, start=True, stop=False)
                for li in range(2):
                    # t0 part: rhs[(n'd), (r, j)] = x[n', d, 24*li + i0 + r]
                    rhs = x2_sb[:, HALF * li + i0:HALF * li + i0 + ci]
                    rhs = rhs[:, :, None].broadcast_to((K, ci, M))
                    nc.tensor.matmul(blk, w0_l(li), rhs,
                                     start=False, stop=(li == 1))

            emit_b_chunk(0)

            # ---- T3[q, 24*li + r] (l-masked diag values, separate halves)
            T3_ps = ppool.tile([P, 2 * HALF], f32)
            for li in range(2):
                nc.tensor.matmul(T3_ps[:, li * HALF:(li + 1) * HALF],
                                 w3_l(li),
                                 x2_sb[:, HALF * li:HALF * (li + 1)],
                                 start=True, stop=True)
            T3G = cpool.tile([P, 2 * HALF], f32)
            # on ACT, off the DVE critical path (GpSimd cannot read PSUM)
            nc.scalar.activation(T3G[:], T3_ps[:],
                                 mybir.ActivationFunctionType.Copy)

            # ---- main loop over row pieces: y row p = 24*q + r ----
            y_v = y_d.rearrange("(q r) f -> q r f", q=P)

            def chunk_of(r0):
                c = max(i for i, s in enumerate(CHUNK_R0) if s <= r0)
                return c, r0 - CHUNK_R0[c]

            # half-parity free step: position li=1 sits 24*49 = 1176 after
            # li=0 within a row, and its T3G column sits HALF=24 later
            HSTEP = HALF * (M + 1)

            def emit_piece(r0, c0, cw, eng, dma_eng):
                out_t = opool.tile([P, cw], f32, tag="out")
                j0, jw = c0 // M, cw // M
                o3 = out_t.rearrange("q (j k) -> q j k", j=jw)
                c, ro = chunk_of(r0)
                B3 = B_chunks[c].rearrange("q (r j) -> q r j", j=M)
                in_j = B3[:, ro, j0:j0 + jw]
                # GpSimd runs a few pieces (~2x slower but parallel) to keep
                # DVE below the stream rate; it cannot read PSUM, so its B
                # slice is mirrored to SBUF by ACT first
                if eng is nc.gpsimd:
                    B_sb = bpool.tile([P, jw], f32, tag="bsb")
                    nc.scalar.activation(B_sb[:], in_j,
                                         mybir.ActivationFunctionType.Copy)
                    in_j = B_sb[:]
                in_j = in_j[:, :, None].broadcast_to((P, jw, M))
                in_k = T2G[:, None, :].broadcast_to((P, jw, M))
                eng.tensor_add(out=o3, in0=in_j, in1=in_k)
                # superdiagonal: add BOTH parity candidate positions on all
                # partitions; the l-masked T3G halves zero the wrong one.
                # Full rows take ONE [P,2] add (positions r0*49, r0*49+1176
                # paired with T3G cols r0, r0+24); half pieces hold only one
                # candidate position
                g0 = r0 * (M + 1) - c0
                if 0 <= g0 and g0 + HSTEP < cw:
                    dv = out_t[:, g0::HSTEP][:, :2]
                    nc.gpsimd.tensor_add(
                        out=dv, in0=dv, in1=T3G[:, r0::HALF][:, :2])
                else:
                    for li in range(2):
                        g = g0 + li * HSTEP
                        if 0 <= g < cw:
                            dv = out_t[:, g:g + 1]
                            nc.gpsimd.tensor_add(
                                out=dv, in0=dv,
                                in1=T3G[:, li * HALF + r0:li * HALF + r0 + 1])
                dma_eng.dma_start(out=y_v[:, r0, c0:c0 + cw], in_=out_t[:])

            V, G = nc.vector, nc.gpsimd
            SY, SC = nc.sync, nc.scalar
            HJK = JK // 2
            # (r0, c0, cw, compute engine, dma ring): single-row pieces in
            # strict ring alternation; first and last rows split in half so
            # both rings start early and end with small transfers; GpSimd
            # carries every 4th row so DVE keeps pace even when the chip
            # throttles compute clocks (~20% run-to-run). 12 rows per ring.
            schedule = [(0, 0, HJK, V, SY), (0, HJK, HJK, V, SC)]
            g_rows = {3, 6, 10, 13, 17, 20}
            for r in range(1, HALF - 1):
                schedule.append(
                    (r, 0, JK, G if r in g_rows else V, SY if r % 2 else SC))
            schedule += [(23, 0, HJK, V, SY), (23, HJK, HJK, V, SC)]
            next_chunk = 1
            # keep the PE one chunk ahead of the piece being computed
            for (r0, c0, cw, eng, dma_eng) in schedule:
                while (next_chunk < len(CHUNKS)
                       and CHUNK_R0[next_chunk] <= r0 + 2):
                    emit_b_chunk(next_chunk)
                    next_chunk += 1
                emit_piece(r0, c0, cw, eng, dma_eng)
            while next_chunk < len(CHUNKS):
                emit_b_chunk(next_chunk)
                next_chunk += 1

    nc.compile()
    return nc


def _get_prog():
    global _PROG
    if _PROG is None:
        _PROG = _build_prog()
    return _PROG


def _make_in_maps(x, coefs, bias):
    import ml_dtypes

    x = np.asarray(x, dtype=np.float32)
    coefs = np.asarray(coefs, dtype=np.float32)
    bias = np.asarray(bias, dtype=np.float32)

    # partition q: ns(q) = q//2 = n*32 + s;  l(q) = q%2
    q = np.arange(P)
    n_of = q // 2 // S
    s_of = q // 2 % S
    # indicator weights w_b[(n',d), q] = coefs[d, s(q), b] * (n' == n(q))
    nd_n = np.repeat(np.arange(NL), D)                # (K,) n' of row
    nd_d = np.tile(np.arange(D), NL)                  # (K,) d of row
    sel = (nd_n[:, None] == n_of[None, :]).astype(np.float32)  # (K, P)

    def w_of(b):
        return coefs[nd_d[:, None], s_of[None, :], b] * sel

    lmask = [((q % 2) == li).astype(np.float32)[None, :] for li in range(2)]

    wc = np.zeros((K, WC_COLS), np.float32)
    wc[:, 0 * P:1 * P] = w_of(1)
    wc[:, 1 * P:2 * P] = w_of(2)
    for li in range(2):
        wc[:, (2 + li) * P:(3 + li) * P] = w_of(0) * lmask[li]

    wt = np.zeros((K, WT_COLS), np.float32)
    for li in range(2):
        wt[:, li * P:(li + 1) * P] = w_of(3) * lmask[li]

    bv = bias.reshape(S)[s_of].reshape(P, 1).astype(np.float32)

    wt_b = np.ascontiguousarray(wt.astype(ml_dtypes.bfloat16))
    bv_c = np.ascontiguousarray(bv)

    in_maps = []
    for core in range(N_CORES):
        x2 = x[NL * core:NL * (core + 1)].reshape(K, M)
        wc_full = wc.copy()
        wc_full[:, 4 * P:4 * P + M] = x2
        wc_b = np.ascontiguousarray(wc_full.astype(ml_dtypes.bfloat16))
        in_maps.append({"wc": wc_b, "wt": wt_b, "bv": bv_c})
    return in_maps


def run(x, coefs, bias, **run_kwargs):
    """Run on hardware; returns (full_output, BassKernelResults)."""
    from concourse.bass_utils import run_bass_kernel_spmd

    prog = _get_prog()
    in_maps = _make_in_maps(x, coefs, bias)
    res = run_bass_kernel_spmd(prog, in_maps, list(range(N_CORES)), **run_kwargs)
    out = np.concatenate(
        [res.results[i]["y"].reshape(NL, S, M, M, M) for i in range(N_CORES)],
        axis=0)
    return out, res


def kernel(x, coefs, bias):
    out, _ = run(x, coefs, bias)
    return out


# revision 11
# speedup vs baseline: 1.1576x; 1.1576x over previous
"""Trainium2 Bass kernel for nn_Eq1to3 (eset_ops_1_to_3 + einsum broadcast expansion).

Reference computation (N=16, D=64, S=32, M=48, BASIS=4):
    t[b,n,s,m] = sum_d coefs[d,s,b] * x[n,d,m]        # tiny einsum
    out[n,s,i,j,k] = t0[n,s,i] + t1[n,s,j] + t2[n,s,k]
                     + (i==j==k) * t3[n,s,i] + bias[s]
Output (16, 32, 48, 48, 48) f32 = 226.5 MB -> HBM-write-bound.
Measured: each HWDGE ring sustains ~220 GB/s on full 128-partition
transfers; both rings together ~440 GB/s per core, so the per-core stream
floor is 28.3 MB / 440 GB/s ~= 65 us, plus ~10 us fixed startup (Tile
preamble + load-completion receipt) and a ~2.5 us tail. (64-partition
half-DMAs only engage 8 of the 16 SDMA engines and halve the ring rate;
partition-strided APs are rejected by the BIR verifier -- both measured.)

Strategy: data-parallel over N across 8 cores (2 batches/core). Per core the
output is [3072 rows p=(n,s,i), 2304 cols (j,k)]. Partition q holds the 24
consecutive rows p = 24*q + r, so ns(q) = q//2 and i(q,r) = 24*(q%2) + r.

All tables are produced by the TENSOR ENGINE in bf16 (single-pass matmuls,
fp32 PSUM accumulate; rel err ~2e-3 << the 2e-2 gate) from host-prepared
indicator weights lhsT[(n',d), q] = coefs[d, s(q), b] * (n'==n(q)) against
rhs slices/broadcasts of x2[(n,d), m] = x[n,d,m]:

    B_ps[q, (r,j)]      = t0[ns(q), i(q,r)] + t1[ns(q), j]  (3 matmuls/chunk)
    T2_ps[q, k]         = t2[ns(q), k]                       (1 matmul)
    T3_ps[q, 24*li + r] = t3[ns(q), 24*li+r] * (q%2 == li)   (2 l-masked
                          matmuls kept in SEPARATE column halves)

T2G = T2_ps + bias[s(q)] (one tiny DVE add folds the bias; no bias matmuls).
Main loop per piece of rows [r0, r0+rw): DVE (or GpSimd for a few pieces,
with an ACT PSUM->SBUF mirror of B) computes
out[q,(u,j,k)] = B[q,(u,j)] + T2G[q,k] via stride-0 broadcast APs. The
superdiagonal needs out[q, u*2304 + i*49] += t3 at the parity-dependent
i = 24*(q%2)+r0+u, which no legal AP can address directly; instead BOTH
candidate positions get a tiny stepped-AP add on all 128 partitions
(free offsets u*2353 + (24*li + r0)*49, li = 0,1) with values from the
l-MASKED T3G halves -- the zeros cancel the wrong-parity position, so no
one-hot mask tensor, no 590KB mask load, no GpSimd DGM multiply. One
contiguous 128-partition DMA per piece (rows 24q+r0 .. +rw per partition),
alternating the SP/ACT HWDGE rings; first and last rows go out as
single-row pieces so the stream starts early and ends with small transfers.
"""

import numpy as np

N, D, S, M, BASIS = 16, 64, 32, 48, 4
N_CORES = 8
NL = N // N_CORES              # batches per core (2)
NS = NL * S                    # (n,s) groups per core (64)
ROWS = NS * M                  # output rows per core (3072)
JK = M * M                     # free size per row (2304)
P = 128                        # partitions
HALF = M // 2                  # rows per partition (24)
K = NL * D                     # contraction size (128)

# B-matmul chunks in r-rows; each chunk gets its own PSUM bank (a PE-write
# and a DVE-read in the same PSUM bank is a hardware fault, so chunks must
# not share banks while the main loop streams from an earlier chunk).
# 6 B chunks + T2 + T3 = 8 PSUM tiles = all 8 banks.
CHUNKS = [1, 1, 2, 4, 8, 8]
CHUNK_R0 = [sum(CHUNKS[:c]) for c in range(len(CHUNKS))]

# critical-load column blocks (bf16): 0=t1, 1=t2, 2=t0l0, 3=t0l1, then x2
WC_COLS = 4 * P + M            # [K, 560]
# second load: 0=t3l0, 1=t3l1 (bf16)
WT_COLS = 2 * P

_PROG = None


def _build_prog():
    import concourse.bacc as bacc
    import concourse.tile as tile
    import concourse.mybir as mybir

    f32 = mybir.dt.float32
    bf16 = mybir.dt.bfloat16
    nc = bacc.Bacc("TRN2", target_bir_lowering=False, debug=False,
                   num_devices=N_CORES)

    wc_d = nc.dram_tensor("wc", [K, WC_COLS], bf16,
                          kind="ExternalInput").ap()
    wt_d = nc.dram_tensor("wt", [K, WT_COLS], bf16,
                          kind="ExternalInput").ap()
    bv_d = nc.dram_tensor("bv", [P, 1], f32, kind="ExternalInput").ap()
    y_d = nc.dram_tensor("y", [ROWS, JK], f32, kind="ExternalOutput").ap()

    with tile.TileContext(nc) as tc:
        with (
            tc.tile_pool(name="const", bufs=1) as cpool,
            tc.tile_pool(name="psum", bufs=1, space="PSUM") as ppool,
            tc.tile_pool(name="outp", bufs=8) as opool,
            tc.tile_pool(name="bsb", bufs=3) as bpool,
        ):
            # ---- loads: critical weights on SP ring, the rest on ACT ring
            wc_sb = cpool.tile([K, WC_COLS], bf16)
            nc.sync.dma_start(out=wc_sb[:], in_=wc_d[:])
            bv_sb = cpool.tile([P, 1], f32)
            nc.scalar.dma_start(out=bv_sb[:], in_=bv_d[:])
            wt_sb = cpool.tile([K, WT_COLS], bf16)
            nc.scalar.dma_start(out=wt_sb[:], in_=wt_d[:])

            w1_l = wc_sb[:, 0 * P:1 * P]
            w2_l = wc_sb[:, 1 * P:2 * P]
            w0_l = lambda li: wc_sb[:, (2 + li) * P:(3 + li) * P]
            x2_sb = wc_sb[:, 4 * P:4 * P + M]
            w3_l = lambda li: wt_sb[:, li * P:(li + 1) * P]

            # ---- T2 first (gates every output piece) ----
            T2_ps = ppool.tile([P, M], f32)
            nc.tensor.matmul(T2_ps[:], w2_l, x2_sb, start=True, stop=True)
            T2G = cpool.tile([P, M], f32)
            # fold bias in while copying PSUM->SBUF
            nc.vector.tensor_add(out=T2G[:], in0=T2_ps[:],
                                 in1=bv_sb[:, 0:1].broadcast_to((P, M)))

            # ---- B[q, (r, j)] via accumulating matmuls ----
            B_chunks = [ppool.tile([P, ci * M], f32, name=f"B_ps{c}")
                        for c, ci in enumerate(CHUNKS)]

            def emit_b_chunk(c):
                ci = CHUNKS[c]
                i0 = CHUNK_R0[c]
                blk = B_chunks[c].rearrange("q (r j) -> q r j", j=M)
                # t1 part: rhs[(n'd), (r, j)] = x[n', d, j]
                rhs = x2_sb[:, None, :].broadcast_to((K, ci, M))
                nc.tensor.matmul(blk, w1_l, rhs, start=True, stop=False)
                for li in range(2):
                    # t0 part: rhs[(n'd), (r, j)] = x[n', d, 24*li + i0 + r]
                    rhs = x2_sb[:, HALF * li + i0:HALF * li + i0 + ci]
                    rhs = rhs[:, :, None].broadcast_to((K, ci, M))
                    nc.tensor.matmul(blk, w0_l(li), rhs,
                                     start=False, stop=(li == 1))

            emit_b_chunk(0)

            # ---- T3[q, 24*li + r] (l-masked diag values, separate halves)
            T3_ps = ppool.tile([P, 2 * HALF], f32)
            for li in range(2):
                nc.tensor.matmul(T3_ps[:, li * HALF:(li + 1) * HALF],
                                 w3_l(li),
                                 x2_sb[:, HALF * li:HALF * (li + 1)],
                                 start=True, stop=True)
            T3G = cpool.tile([P, 2 * HALF], f32)
            # on ACT, off the DVE critical path (GpSimd cannot read PSUM)
            nc.scalar.activation(T3G[:], T3_ps[:],
                                 mybir.ActivationFunctionType.Copy)

            # ---- main loop over row pieces: y row p = 24*q + r ----
            y_v = y_d.rearrange("(q r) f -> q r f", q=P)

            def chunk_of(r0):
                c = max(i for i, s in enumerate(CHUNK_R0) if s <= r0)
                return c, r0 - CHUNK_R0[c]

            # half-parity free step: position li=1 sits 24*49 = 1176 after
            # li=0 within a row, and its T3G column sits HALF=24 later
            HSTEP = HALF * (M + 1)

            def emit_piece(r0, c0, cw, eng, dma_eng):
                out_t = opool.tile([P, cw], f32, tag="out")
                j0, jw = c0 // M, cw // M
                o3 = out_t.rearrange("q (j k) -> q j k", j=jw)
                c, ro = chunk_of(r0)
                B3 = B_chunks[c].rearrange("q (r j) -> q r j", j=M)
                in_j = B3[:, ro, j0:j0 + jw]
                # GpSimd runs a few pieces (~2x slower but parallel) to keep
                # DVE below the stream rate; it cannot read PSUM, so its B
                # slice is mirrored to SBUF by ACT first
                if eng is nc.gpsimd:
                    B_sb = bpool.tile([P, jw], f32, tag="bsb")
                    nc.scalar.activation(B_sb[:], in_j,
                                         mybir.ActivationFunctionType.Copy)
                    in_j = B_sb[:]
                in_j = in_j[:, :, None].broadcast_to((P, jw, M))
                in_k = T2G[:, None, :].broadcast_to((P, jw, M))
                eng.tensor_add(out=o3, in0=in_j, in1=in_k)
                # superdiagonal: add BOTH parity candidate positions on all
                # partitions; the l-masked T3G halves zero the wrong one.
                # Full rows take ONE [P,2] add (positions r0*49, r0*49+1176
                # paired with T3G cols r0, r0+24); half pieces hold only one
                # candidate position
                g0 = r0 * (M + 1) - c0
                if 0 <= g0 and g0 + HSTEP < cw:
                    dv = out_t[:, g0::HSTEP][:, :2]
                    nc.gpsimd.tensor_add(
                        out=dv, in0=dv, in1=T3G[:, r0::HALF][:, :2])
                else:
                    for li in range(2):
                        g = g0 + li * HSTEP
                        if 0 <= g < cw:
                            dv = out_t[:, g:g + 1]
                            nc.gpsimd.tensor_add(
                                out=dv, in0=dv,
                                in1=T3G[:, li * HALF + r0:li * HALF + r0 + 1])
                dma_eng.dma_start(out=y_v[:, r0, c0:c0 + cw], in_=out_t[:])

            V, G = nc.vector, nc.gpsimd
            HJK, QJK = JK // 2, JK // 4
            # (r0, c0, cw, compute engine): single-row pieces; row 0 starts
            # as two quarter pieces so both rings begin streaming as early
            # as possible, row 23 ends as halves so the stream finishes with
            # small transfers; GpSimd carries every 4th row so DVE keeps
            # pace even when the chip throttles compute clocks (~20%
            # run-to-run)
            schedule = [(0, 0, QJK, V), (0, QJK, QJK, V), (0, HJK, HJK, V)]
            g_rows = {3, 6, 10, 13, 17, 20}
            for r in range(1, HALF - 1):
                schedule.append((r, 0, JK, G if r in g_rows else V))
            schedule += [(23, 0, HJK, V), (23, HJK, HJK, V)]
            # greedy ring balance: each piece goes to the ring with fewer
            # queued bytes (ties to SP), so both rings stay in lockstep and
            # carry 12 rows each
            ring_bytes = {0: 0, 1: 0}
            rings = (nc.sync, nc.scalar)
            next_chunk = 1
            # keep the PE one chunk ahead of the piece being computed
            for (r0, c0, cw, eng) in schedule:
                while (next_chunk < len(CHUNKS)
                       and CHUNK_R0[next_chunk] <= r0 + 2):
                    emit_b_chunk(next_chunk)
                    next_chunk += 1
                ri = 0 if ring_bytes[0] <= ring_bytes[1] else 1
                ring_bytes[ri] += cw
                emit_piece(r0, c0, cw, eng, rings[ri])
            while next_chunk < len(CHUNKS):
                emit_b_chunk(next_chunk)
                next_chunk += 1

    nc.compile()
    return nc


def _get_prog():
    global _PROG
    if _PROG is None:
        _PROG = _build_prog()
    return _PROG


def _make_in_maps(x, coefs, bias):
    import ml_dtypes

    x = np.asarray(x, dtype=np.float32)
    coefs = np.asarray(coefs, dtype=np.float32)
    bias = np.asarray(bias, dtype=np.float32)

    # partition q: ns(q) = q//2 = n*32 + s;  l(q) = q%2
    q = np.arange(P)
    n_of = q // 2 // S
    s_of = q // 2 % S
    # indicator weights w_b[(n',d), q] = coefs[d, s(q), b] * (n' == n(q))
    nd_n = np.repeat(np.arange(NL), D)                # (K,) n' of row
    nd_d = np.tile(np.arange(D), NL)                  # (K,) d of row
    sel = (nd_n[:, None] == n_of[None, :]).astype(np.float32)  # (K, P)

    def w_of(b):
        return coefs[nd_d[:, None], s_of[None, :], b] * sel

    lmask = [((q % 2) == li).astype(np.float32)[None, :] for li in range(2)]

    wc = np.zeros((K, WC_COLS), np.float32)
    wc[:, 0 * P:1 * P] = w_of(1)
    wc[:, 1 * P:2 * P] = w_of(2)
    for li in range(2):
        wc[:, (2 + li) * P:(3 + li) * P] = w_of(0) * lmask[li]

    wt = np.zeros((K, WT_COLS), np.float32)
    for li in range(2):
        wt[:, li * P:(li + 1) * P] = w_of(3) * lmask[li]

    bv = bias.reshape(S)[s_of].reshape(P, 1).astype(np.float32)

    wt_b = np.ascontiguousarray(wt.astype(ml_dtypes.bfloat16))
    bv_c = np.ascontiguousarray(bv)

    in_maps = []
    for core in range(N_CORES):
        x2 = x[NL * core:NL * (core + 1)].reshape(K, M)
        wc_full = wc.copy()
        wc_full[:, 4 * P:4 * P + M] = x2
        wc_b = np.ascontiguousarray(wc_full.astype(ml_dtypes.bfloat16))
        in_maps.append({"wc": wc_b, "wt": wt_b, "bv": bv_c})
    return in_maps


def run(x, coefs, bias, **run_kwargs):
    """Run on hardware; returns (full_output, BassKernelResults)."""
    from concourse.bass_utils import run_bass_kernel_spmd

    prog = _get_prog()
    in_maps = _make_in_maps(x, coefs, bias)
    res = run_bass_kernel_spmd(prog, in_maps, list(range(N_CORES)), **run_kwargs)
    out = np.concatenate(
        [res.results[i]["y"].reshape(NL, S, M, M, M) for i in range(N_CORES)],
        axis=0)
    return out, res


def kernel(x, coefs, bias):
    out, _ = run(x, coefs, bias)
    return out
